# revision 24
# baseline (speedup 1.0000x reference)
"""GQA (n_group == n_head) causal attention kernel for 8 Trainium2 NeuronCores.

Sharding: core c -> (batch b = c//2, head-half hh = c%2).  Each core computes
Q/K/V projections for its 8 heads over the full sequence, causal attention,
and a partial output projection against its 512 rows of Wo.  The host sums
the two partial outputs per batch (the tensor-parallel reduce) and
transposes back.

All device matmuls run transposed:
  QT/KT = (x @ W).T        [dout, t]   (lhsT = W chunk, rhs = xT chunk)
  V     = x @ Wv           [t, dout]   (lhsT = xT chunk, rhs = Wv)
  scoresT = K_h @ Q_h.T    [k, q]      (lhsT = KT_h tile, rhs = QT_h block)
  expT  = exp(scoresT/8)               (ScalarE, PSUM -> SBUF)
  pv    = [V_h | 1].T @ expT  [65, q]  (row 64 = softmax denominator)
  outT  = Wo_h.T @ attn_outT  [dout, q] (partial; host adds pairs)
"""

import os
from contextlib import ExitStack

import numpy as np

import concourse.bass as bass
import concourse.mybir as mybir
import concourse.tile as tile
from concourse import bacc
from concourse.bass import ds, ts
from concourse.bass_utils import run_bass_kernel_spmd

B, T, D = 4, 2048, 1024
H, HD = 16, 64
NCORES = 8
HH = H // 2            # heads per core = 8
DH = HH * HD           # head dims per core = 512
QC = 512               # query chunk (free dim of attention matmuls)
NQC = T // QC          # 4 query chunks
KT_TILE = 128          # key tile (contraction dim of PV matmul)
TB = 512               # token block for projections
F32 = mybir.dt.float32

# matmul input dtype mode: "f32" (exact, 4 cyc/row) or "f32r" (fast, 1 cyc/row)
MM_MODE = os.environ.get("KERNEL_MM_MODE", "f32r")
MF = mybir.dt.float32r if MM_MODE == "f32r" else mybir.dt.float32

LAST_RESULTS = None


def _build_nc():
    nc = bacc.Bacc(
        "TRN2",
        target_bir_lowering=False,
        debug=False,
        enable_asserts=False,
        num_devices=NCORES,
    )

    xT = nc.dram_tensor("xT", [D, T], MF, kind="ExternalInput").ap()
    wq = nc.dram_tensor("wq", [D, DH], MF, kind="ExternalInput").ap()
    wk = nc.dram_tensor("wk", [D, DH], MF, kind="ExternalInput").ap()
    wv = nc.dram_tensor("wv", [D, DH], MF, kind="ExternalInput").ap()
    wo = nc.dram_tensor("wo", [DH, D], MF, kind="ExternalInput").ap()
    bq = nc.dram_tensor("bq", [1, DH], MF, kind="ExternalInput").ap()
    bk = nc.dram_tensor("bk", [1, DH], MF, kind="ExternalInput").ap()
    bv = nc.dram_tensor("bv", [1, DH], MF, kind="ExternalInput").ap()
    bo_t = nc.dram_tensor("bo_t", [128, D // 128], F32, kind="ExternalInput").ap()
    masks = nc.dram_tensor("masks", [128, 4 * QC], mybir.dt.bfloat16, kind="ExternalInput").ap()
    ones_in = nc.dram_tensor("ones_in", [128, TB], MF, kind="ExternalInput").ap()
    outT = nc.dram_tensor("outT", [D, T], F32, kind="ExternalOutput").ap()

    with tile.TileContext(nc) as tc, ExitStack() as ctx:
        if True:
            res = ctx.enter_context(tc.tile_pool(name="res", bufs=1))
            # resident SBUF tensors
            qt_sb = res.tile([128, 4, T], MF, tag="qt")      # QT: row c*128+p = local dout
            kt_sb = res.tile([128, 4, T], MF, tag="kt")
            v_sb = res.tile([128, T // 128, HH, HD + 1], MF, tag="v")  # [p, ktile, h, hd|1]
            ao_sb = res.tile([128, 4, T], MF, tag="ao")      # attn_outT
            mask_sb = res.tile([128, 4, QC], mybir.dt.bfloat16, tag="mask")
            ones_sb = res.tile([1, TB], MF, tag="ones")
            bq_sb = res.tile([1, DH], MF, tag="bq")
            bk_sb = res.tile([1, DH], MF, tag="bk")
            bv_sb = res.tile([1, DH], MF, tag="bv")
            bo_sb = res.tile([128, D // 128], F32, tag="bo")

            nc.sync.dma_start(out=mask_sb, in_=masks.rearrange("p (j q) -> p j q", q=QC))
            nc.sync.dma_start(out=ones_sb, in_=ones_in[0:1, :])
            nc.sync.dma_start(out=bq_sb, in_=bq)
            nc.sync.dma_start(out=bk_sb, in_=bk)
            nc.sync.dma_start(out=bv_sb, in_=bv)
            nc.sync.dma_start(out=bo_sb, in_=bo_t)
            # ones column of V~ (softmax denominator accumulator)
            nc.sync.dma_start(
                out=v_sb[:, :, :, HD : HD + 1],
                in_=ones_in[:, 0:128].rearrange("p (a b c) -> p a b c", b=HH, c=1),
            )

            psum = ctx.enter_context(tc.tile_pool(name="psum", bufs=2, space="PSUM"))
            scpsum = ctx.enter_context(tc.tile_pool(name="scpsum", bufs=3, space="PSUM"))
            pvpsum = ctx.enter_context(tc.tile_pool(name="pvpsum", bufs=3, space="PSUM"))

            if True:
                # ---------------- projections ----------------
                with tc.tile_pool(name="wproj", bufs=1) as wpool, tc.tile_pool(
                    name="xstream", bufs=2
                ) as xpool:
                    for phase in ("q", "kv"):
                        if phase == "q":
                            w_sb = wpool.tile([128, 8, DH], MF, tag="w1")
                            for c in range(8):
                                nc.sync.dma_start(out=w_sb[:, c, :], in_=wq[ts(c, 128), :])
                        else:
                            w_sb = wpool.tile([128, 8, DH], MF, tag="w1")
                            wv_sb = wpool.tile([128, 8, DH], MF, tag="w2")
                            for c in range(8):
                                nc.sync.dma_start(out=w_sb[:, c, :], in_=wk[ts(c, 128), :])
                                nc.sync.dma_start(out=wv_sb[:, c, :], in_=wv[ts(c, 128), :])
                        b_sb = bq_sb if phase == "q" else bk_sb
                        dst = qt_sb if phase == "q" else kt_sb
                        for tb in range(T // TB):
                            xt = xpool.tile([128, 8, TB], MF, tag="xt")
                            for c in range(8):
                                nc.sync.dma_start(
                                    out=xt[:, c, :],
                                    in_=xT[ts(c, 128), ts(tb, TB)],
                                )
                            # QT / KT: [dout, t] blocks
                            for dt in range(DH // 128):
                                ps = psum.tile([128, TB], F32, tag="ps")
                                for c in range(8):
                                    nc.tensor.matmul(
                                        ps,
                                        w_sb[:, c, ts(dt, 128)],
                                        xt[:, c, :],
                                        start=(c == 0),
                                        stop=False,
                                    )
                                nc.tensor.matmul(
                                    ps,
                                    b_sb[0:1, ts(dt, 128)],
                                    ones_sb[0:1, :],
                                    start=False,
                                    stop=True,
                                )
                                nc.vector.tensor_copy(dst[:, dt, ts(tb, TB)], ps)
                            if phase == "kv":
                                # V: [t, dout] tiles (same xt tile)
                                for tt in range(TB // 128):
                                    ps = psum.tile([128, DH], F32, tag="ps")
                                    for c in range(8):
                                        nc.tensor.matmul(
                                            ps,
                                            xt[:, c, ts(tt, 128)],
                                            wv_sb[:, c, :],
                                            start=(c == 0),
                                            stop=False,
                                        )
                                    nc.tensor.matmul(
                                        ps,
                                        ones_sb[0:1, 0:128],
                                        bv_sb[0:1, :],
                                        start=False,
                                        stop=True,
                                    )
                                    kt_g = tb * (TB // 128) + tt
                                    nc.vector.tensor_copy(
                                        v_sb[:, kt_g, :, 0:HD],
                                        ps.rearrange("p (h c) -> p h c", h=HH),
                                    )

                # ---------------- attention ----------------
                with tc.tile_pool(name="att", bufs=5) as att, tc.tile_pool(
                    name="attn2", bufs=2
                ) as att2, tc.tile_pool(name="oproj", bufs=1) as opool, tc.tile_pool(
                    name="ostage", bufs=4
                ) as ostage:
                    wo_sb = opool.tile([128, 4, D], MF, tag="wo")
                    for c in range(4):
                        nc.sync.dma_start(out=wo_sb[:, c, :], in_=wo[ts(c, 128), :])
                    for qc in range(NQC):
                        for hp in range(HH // 2):  # head pair: heads 2hp, 2hp+1
                            ch = hp                # chunk index in qt/kt layout
                            nkt = (qc + 1) * (QC // KT_TILE)
                            pv0 = pvpsum.tile([HD + 1, QC], F32, tag="pv")
                            pv1 = pvpsum.tile([HD + 1, QC], F32, tag="pv")
                            pvs = [pv0, pv1]
                            for kt in range(nkt):
                                j = kt - qc * (QC // KT_TILE)
                                # diag tile j: only q >= j*128 is unmasked
                                q0 = j * KT_TILE if j > 0 else 0
                                qn = QC - q0
                                ets = []
                                for i in range(2):  # QK for both heads, back-to-back
                                    po = i * 64
                                    sc = scpsum.tile([128, QC], F32, tag="sc")
                                    nc.tensor.matmul(
                                        sc[:, q0:],
                                        kt_sb[po : po + 64, ch, ts(kt, 128)],
                                        qt_sb[po : po + 64, ch, ds(qc * QC + q0, qn)],
                                        start=True,
                                        stop=True,
                                    )
                                    et = att.tile([128, QC], MF, tag="et")
                                    nc.scalar.activation(
                                        et[:, q0:],
                                        sc[:, q0:],
                                        mybir.ActivationFunctionType.Exp,
                                        scale=0.125,
                                    )
                                    if j >= 0:  # diagonal tile: apply causal mask
                                        nc.vector.tensor_mul(
                                            et[:, q0:], et[:, q0:], mask_sb[:, j, q0:]
                                        )
                                    ets.append(et)
                                for i in range(2):
                                    nc.tensor.matmul(
                                        pvs[i][:, q0:],
                                        v_sb[:, kt, 2 * hp + i, :],
                                        ets[i][:, q0:],
                                        start=(kt == 0),
                                        stop=(kt == nkt - 1),
                                    )
                            # normalize: rows 0..63 / row 64
                            for i in range(2):
                                po = i * 64
                                pv = pvs[i]
                                rd = att2.tile([1, QC], F32, tag="rd")
                                nc.vector.reciprocal(rd, pv[HD : HD + 1, :])
                                bcs = att2.tile([64, QC], F32, tag="bcs")
                                rd_bcast = bass.AP(
                                    tensor=rd.tensor,
                                    offset=rd.offset,
                                    ap=[rd.ap[0], [0, 64]] + rd.ap[1:],
                                )
                                nc.sync.dma_start(out=bcs, in_=rd_bcast)
                                nc.vector.tensor_mul(
                                    ao_sb[po : po + 64, ch, ts(qc, QC)], pv[0:HD, :], bcs
                                )
                        # output projection for this q block (partial over my heads)
                        for dt in range(D // 128):
                            ps = psum.tile([128, TB], F32, tag="ps")
                            for c in range(4):
                                nc.tensor.matmul(
                                    ps,
                                    wo_sb[:, c, ts(dt, 128)],
                                    ao_sb[:, c, ts(qc, TB)],
                                    start=(c == 0),
                                    stop=(c == 3),
                                )
                            st = ostage.tile([128, TB], F32, tag="st")
                            nc.vector.tensor_scalar_add(st, ps, bo_sb[:, dt : dt + 1])
                            nc.sync.dma_start(out=outT[ts(dt, 128), ts(qc, TB)], in_=st)


    nc.compile()
    return nc


def _make_masks():
    k = np.arange(128)[:, None]
    q = np.arange(QC)[None, :]
    m = np.zeros((128, 4, QC), np.float32)
    for j in range(4):
        m[:, j, :] = (q >= j * KT_TILE + k).astype(np.float32)
    return np.ascontiguousarray(m.reshape(128, 4 * QC))


def kernel(x, Wq, bq, Wk, bk, Wv, bv, Wo, bo):
    global LAST_RESULTS
    x = np.asarray(x, np.float32)
    Wq, bq = np.asarray(Wq, np.float32), np.asarray(bq, np.float32)
    Wk, bk = np.asarray(Wk, np.float32), np.asarray(bk, np.float32)
    Wv, bv = np.asarray(Wv, np.float32), np.asarray(bv, np.float32)
    Wo, bo = np.asarray(Wo, np.float32), np.asarray(bo, np.float32)

    import ml_dtypes
    masks = _make_masks().astype(ml_dtypes.bfloat16)
    ones = np.ones((128, TB), np.float32)
    bo_t = np.ascontiguousarray(bo.reshape(D // 128, 128).T)  # [128, 8]
    zeros_bo = np.zeros_like(bo_t)

    in_maps = []
    for c in range(NCORES):
        b, hh = c // 2, c % 2
        sl = slice(hh * DH, (hh + 1) * DH)
        in_maps.append(
            {
                "xT": np.ascontiguousarray(x[b].T),
                "wq": np.ascontiguousarray(Wq[:, sl]),
                "wk": np.ascontiguousarray(Wk[:, sl]),
                "wv": np.ascontiguousarray(Wv[:, sl]),
                "wo": np.ascontiguousarray(Wo[sl, :]),
                "bq": np.ascontiguousarray(bq[sl]).reshape(1, DH),
                "bk": np.ascontiguousarray(bk[sl]).reshape(1, DH),
                "bv": np.ascontiguousarray(bv[sl]).reshape(1, DH),
                "bo_t": bo_t if hh == 0 else zeros_bo,
                "masks": masks,
                "ones_in": ones,
            }
        )

    nc = _build_nc()
    res = run_bass_kernel_spmd(
        nc,
        in_maps,
        core_ids=list(range(NCORES)),
        trace=bool(int(os.environ.get("KERNEL_TRACE", "0"))),
    )
    LAST_RESULTS = res

    out = np.empty((B, T, D), np.float32)
    for b in range(B):
        acc = res.results[2 * b]["outT"] + res.results[2 * b + 1]["outT"]
        out[b] = acc.T
    return out
